# revision 17
# baseline (speedup 1.0000x reference)
"""Pairwise cosine-similarity kernel for Trainium2 (8 NeuronCores, SPMD).

Computes out = 16 * normalize(x1) @ normalize(x2).T for x1, x2 [8192, 512] f32.

Sharding: x1 rows are split across the 8 cores (1024 rows each); x2 is
replicated. Each core computes its [1024, 8192] slice of the output; the host
concatenates the slices.

Host-side prep is layout/dtype only: inputs are cast to bf16; both GEMM
operands ship pre-transposed and K-chunk-blocked ([128, 4, cols], contiguous
per partition) so they load straight into SBUF in GEMM layout with no
on-device transposition; x1 additionally ships in natural layout (1 MB) for
row-norm stats. The device writes the output in bf16 and the host widens it
to f32 (exact). All FLOPs run on device:

  1. x1 row norms from the natural copy: fused Square+row-sum on ScalarE ->
     sqrt -> DVE reciprocal -> inv1 [128, 8] f32 (16/|row| for row-tile m at
     column m, partition p = local row m*128+p).
  2. x2 column norms per 1024-wide column group: Square (ScalarE) -> pairwise
     K-chunk adds (DVE) -> ones.T @ ssum on the PE (one matmul reduces over
     the partition dim AND broadcasts to all 128 partitions) -> sqrt (ScalarE)
     -> reciprocal_approx_fast (DVE) -> inv2 [128, 1024] f32.
  3. Main GEMM: out_tile[128, 1024] += x1T.T @ x2T over 4 K-chunks (bf16,
     f32 PSUM). Normalization is folded into the PSUM drain: for cg 0-1
     (warmup, x2T raw) a DVE scalar_tensor_tensor computes
     (psum * inv1[m]) * inv2; for cg 2-7 inv2 is pre-multiplied into x2T
     (one DVE pass per cg, done with >25us of pipeline lookahead) and the
     drain is (psum * inv1[m]) alternating DVE tensor_scalar / ScalarE
     copy-with-scale.

DMA rings: Sync carries x1-natural + x2 cg0 + all output stores; Scalar
carries x1T + x2 cg1-7. HBM traffic per core: 10 MB in + 16 MB out.
"""

import sys

for _p in ("/root/.axon_site/_ro/trn_rl_repo", "/opt/trn_rl_repo"):
    if _p not in sys.path:
        sys.path.append(_p)

import ml_dtypes
import numpy as np

import concourse.bass as bass
import concourse.tile as tile
from concourse import bacc, mybir
from concourse.bass_utils import run_bass_kernel_spmd

F32 = mybir.dt.float32
BF16 = mybir.dt.bfloat16
P = 128
SCALE = 16.0
EPS = 1e-8

N_CORES = 8
N1 = 8192  # x1 rows (total)
N2 = 8192  # x2 rows
D = 512  # feature dim
KC = D // P  # K-chunks of the contraction dim
CGW = 1024  # output column-group width

_PROGRAM_CACHE = {}


def build_program(n1_local=N1 // N_CORES, n2=N2, d=D, cg_width=CGW):
    """Build the SPMD program one core runs. Returns the compiled Bacc."""
    kc = d // P
    m_tiles = n1_local // P  # 8 output row-tiles
    n_cgs = n2 // cg_width  # 8 output column groups
    nch = cg_width // 512  # 512-wide PSUM chunks per column group

    nc = bacc.Bacc("TRN2", target_bir_lowering=False, debug=False,
                   num_devices=N_CORES)
    x1n_in = nc.dram_tensor("x1", [n1_local, d], BF16, kind="ExternalInput")
    x1t = nc.dram_tensor("x1t", [P, kc, n1_local], BF16, kind="ExternalInput")
    x2t = nc.dram_tensor("x2t", [n_cgs, P, kc, cg_width], BF16,
                         kind="ExternalInput")
    out = nc.dram_tensor("out", [n1_local, n2], BF16, kind="ExternalOutput")

    with tile.TileContext(nc) as tc:
        with (
            tc.tile_pool(name="const", bufs=1) as const,
            tc.tile_pool(name="xt", bufs=1) as xt,
            tc.tile_pool(name="sq", bufs=2) as sqp,
            tc.tile_pool(name="tmp", bufs=4) as tmp,
            tc.tile_pool(name="stat", bufs=2) as stat,
            tc.tile_pool(name="nrm", bufs=2) as nrmp,
            tc.tile_pool(name="inv", bufs=3) as invp,
            tc.tile_pool(name="outs", bufs=4) as outs,
            tc.tile_pool(name="pso", bufs=3, space="PSUM") as pso,
            tc.tile_pool(name="psa", bufs=2, space="PSUM") as psa,
        ):
            ones_b = const.tile([P, P], BF16)
            nc.gpsimd.memset(ones_b[:], 1.0)

            # x1 natural rows: row = a*128 + p
            x1r = x1n_in.ap().rearrange("(a p) e -> p a e", p=P)

            x1T = xt.tile([P, kc, n1_local], BF16, tag="x1T", name="x1T")
            x2T = [xt.tile([P, kc, cg_width], BF16, tag=f"x2T_{cg}",
                           name=f"x2T_{cg}") for cg in range(n_cgs)]
            x1l = xt.tile([P, m_tiles, d], BF16, tag="x1l", name="x1l")
            invs = [None] * n_cgs
            ssums = [None] * n_cgs

            # ---- input DMAs up front, split across both HWDGE rings ------
            nc.sync.dma_start(x1l[:], x1r)
            nc.scalar.dma_start(x2T[0][:], x2t.ap()[0])
            nc.scalar.dma_start(x1T[:], x1t.ap())
            for cg in range(1, n_cgs):
                nc.scalar.dma_start(x2T[cg][:], x2t.ap()[cg])

            # ---- x2 column norms: squares + K-chunk sums ------------------
            def prep_pre(cg):
                sq_t = sqp.tile([P, kc, cg_width], BF16, tag="sq2",
                                name=f"sq2_{cg}")
                nc.scalar.activation(
                    sq_t[:], x2T[cg][:],
                    mybir.ActivationFunctionType.Square,
                )
                s01 = tmp.tile([P, cg_width], BF16, tag="s01")
                s23 = tmp.tile([P, cg_width], BF16, tag="s23")
                ssum = tmp.tile([P, cg_width], BF16, tag="ssum",
                                name=f"ssum_{cg}")
                nc.vector.tensor_add(s01[:], sq_t[:, 0], sq_t[:, 1])
                nc.vector.tensor_add(s23[:], sq_t[:, 2], sq_t[:, 3])
                nc.vector.tensor_add(ssum[:], s01[:], s23[:])
                ssums[cg] = ssum

            # ---- partition reduce+broadcast, sqrt, reciprocal, and fold
            # ---- inv2 into the operand in place ---------------------------
            def prep_post(cg):
                ssum = ssums[cg]
                inv = invp.tile([P, cg_width], F32, tag="inv",
                                name=f"inv_{cg}")
                for c in range(nch):
                    cs = slice(c * 512, (c + 1) * 512)
                    ps_s = psa.tile([P, 512], F32, tag="psa",
                                    name=f"psn_{cg}_{c}")
                    nc.tensor.matmul(ps_s[:], lhsT=ones_b[:], rhs=ssum[:, cs],
                                     start=True, stop=True)
                    nrm = nrmp.tile([P, 512], F32, tag="nrm",
                                    name=f"nrm_{cg}_{c}")
                    nc.scalar.activation(
                        nrm[:], ps_s[:], mybir.ActivationFunctionType.Sqrt
                    )
                    nc.vector.reciprocal_approx_fast(inv[:, cs], nrm[:])
                invs[cg] = inv
                nc.vector.tensor_mul(
                    x2T[cg][:], x2T[cg][:],
                    inv[:, None, :].to_broadcast((P, kc, cg_width)),
                )

            def x1_stats():
                ssq1 = stat.tile([P, m_tiles], F32, tag="ssq1")
                for a in range(m_tiles):
                    sq_t = tmp.tile([P, d], BF16, tag="sq1")
                    nc.scalar.activation(
                        sq_t[:], x1l[:, a],
                        mybir.ActivationFunctionType.Square,
                        accum_out=ssq1[:, a : a + 1],
                    )
                nrm1 = stat.tile([P, m_tiles], F32, tag="nrm1")
                nc.scalar.activation(
                    nrm1[:], ssq1[:], mybir.ActivationFunctionType.Sqrt,
                    scale=1.0 / (SCALE * SCALE),
                )
                inv1 = stat.tile([P, m_tiles], F32, tag="inv1")
                nc.vector.reciprocal(inv1[:], nrm1[:])
                return inv1

            def gemm_m(cg, m):
                ps = pso.tile([P, cg_width], F32, tag="ps",
                              name=f"ps_{cg}_{m}")
                for k in range(kc):
                    for c in range(nch):
                        nc.tensor.matmul(
                            ps[:, c * 512 : (c + 1) * 512],
                            lhsT=x1T[:, k, m * P : (m + 1) * P],
                            rhs=x2T[cg][:, k, c * 512 : (c + 1) * 512],
                            start=(k == 0), stop=(k == kc - 1),
                        )
                return ps

            def drain_m(cg, m, ps):
                ot = outs.tile([P, cg_width], BF16, tag="ot",
                               name=f"ot_{cg}_{m}")
                s1 = inv1[:, m : m + 1]
                if m < 3:
                    nc.vector.tensor_scalar_mul(ot[:], ps[:], s1)
                else:
                    nc.scalar.activation(
                        ot[:], ps[:], mybir.ActivationFunctionType.Copy,
                        scale=s1,
                    )
                # last group's stores go on the otherwise-idle scalar ring
                # so the final drain isn't serialized behind the backlog
                eng = nc.scalar if cg == n_cgs - 1 else nc.sync
                eng.dma_start(
                    out[m * P : (m + 1) * P,
                        cg * cg_width : (cg + 1) * cg_width],
                    ot[:],
                )

            # Emission schedule. Strict per-engine FIFO => emission order is
            # engine queue order: cg0's square goes to ScalarE before the x1
            # stats burst; each prep_pre lands 2 column groups ahead and its
            # prep_post 1.5 groups ahead (m==4 hook) so the PE never waits on
            # the norm chain or the in-place operand scale.
            prep_pre(0)
            prep_post(0)
            inv1 = x1_stats()
            prep_pre(1)
            for cg in range(n_cgs):
                for m in range(m_tiles):
                    ps = gemm_m(cg, m)
                    drain_m(cg, m, ps)
                    if cg == 0 and m == 2:
                        prep_post(1)
                    if m == 1 and cg + 2 < n_cgs:
                        prep_pre(cg + 2)
                    if m == 4 and cg + 2 < n_cgs:
                        prep_post(cg + 2)

    nc.compile()
    return nc


def _get_program():
    key = "default"
    if key not in _PROGRAM_CACHE:
        _PROGRAM_CACHE[key] = build_program()
    return _PROGRAM_CACHE[key]


def make_in_maps(x1: np.ndarray, x2: np.ndarray) -> list:
    x1 = np.asarray(x1, dtype=np.float32)
    x2 = np.asarray(x2, dtype=np.float32)
    assert x1.shape == (N1, D) and x2.shape == (N2, D), (x1.shape, x2.shape)
    x1_b = x1.astype(ml_dtypes.bfloat16)
    # K-chunk-blocked transposes: t[k*128+p, c] lands at tb[p, k, c], so
    # every load is one DMA of 128 contiguous per-partition rows.
    x2t = x2.astype(ml_dtypes.bfloat16).T
    x2tb = np.ascontiguousarray(
        x2t.reshape(KC, P, N2 // CGW, CGW).transpose(2, 1, 0, 3)
    )
    rows = N1 // N_CORES
    maps = []
    for c in range(N_CORES):
        x1c = np.ascontiguousarray(x1_b[c * rows : (c + 1) * rows])
        x1tb = np.ascontiguousarray(
            x1c.T.reshape(KC, P, rows).transpose(1, 0, 2)
        )
        maps.append({"x1": x1c, "x1t": x1tb, "x2t": x2tb})
    return maps


def kernel(x1: np.ndarray, x2: np.ndarray) -> np.ndarray:
    nc = _get_program()
    in_maps = make_in_maps(x1, x2)
    res = run_bass_kernel_spmd(nc, in_maps, core_ids=list(range(N_CORES)))
    return np.concatenate(
        [res.results[c]["out"] for c in range(N_CORES)], axis=0
    ).astype(np.float32)


if __name__ == "__main__":
    rng = np.random.default_rng(0)
    a = rng.standard_normal((N1, D), dtype=np.float32)
    b = rng.standard_normal((N2, D), dtype=np.float32)
    got = kernel(a, b)
    n1 = np.maximum(np.linalg.norm(a, axis=-1, keepdims=True), EPS)
    n2 = np.maximum(np.linalg.norm(b, axis=-1, keepdims=True), EPS)
    want = SCALE * (a / n1) @ (b / n2).T
    err = np.abs(got - want)
    rel = np.linalg.norm(got - want) / np.linalg.norm(want)
    print(f"max abs err: {err.max():.3e}  rel: {rel:.3e}")


# revision 22
# speedup vs baseline: 1.0820x; 1.0820x over previous
"""Pairwise cosine-similarity kernel for Trainium2 (8 NeuronCores, SPMD).

Computes out = 16 * normalize(x1) @ normalize(x2).T for x1, x2 [8192, 512] f32.

Sharding: x1 rows are split across the 8 cores (1024 rows each); x2 is
replicated. Each core computes its [1024, 8192] slice of the output; the host
concatenates the slices.

Host-side prep is layout/dtype only: inputs are cast to bf16; both GEMM
operands ship pre-transposed and K-chunk-blocked ([128, 4, cols], contiguous
per partition) so they load straight into SBUF in GEMM layout with no
on-device transposition; x1 additionally ships in natural layout (1 MB) for
row-norm stats. The device writes the output in bf16 and the host widens it
to f32 (exact). All FLOPs run on device:

  1. x1 row norms from the natural copy: fused Square+row-sum on ScalarE ->
     sqrt -> DVE reciprocal -> inv1 [128, 8] f32 (16/|row| for row-tile m at
     column m, partition p = local row m*128+p).
  2. x2 column norms per 1024-wide column group: Square (ScalarE) -> pairwise
     K-chunk adds (DVE) -> ones.T @ ssum on the PE (one matmul reduces over
     the partition dim AND broadcasts to all 128 partitions) -> sqrt (ScalarE)
     -> reciprocal_approx_fast (DVE) -> inv2 [128, 1024] f32.
  3. Main GEMM: out_tile[128, 1024] += x1T.T @ x2T over 4 K-chunks (bf16,
     f32 PSUM). Normalization is folded into the PSUM drain: for cg 0-1
     (warmup, x2T raw) a DVE scalar_tensor_tensor computes
     (psum * inv1[m]) * inv2; for cg 2-7 inv2 is pre-multiplied into x2T
     (one DVE pass per cg, done with >25us of pipeline lookahead) and the
     drain is (psum * inv1[m]) alternating DVE tensor_scalar / ScalarE
     copy-with-scale.

DMA rings: Sync carries x1-natural + x2 cg0 + all output stores; Scalar
carries x1T + x2 cg1-7. HBM traffic per core: 10 MB in + 16 MB out.
"""

import sys

for _p in ("/root/.axon_site/_ro/trn_rl_repo", "/opt/trn_rl_repo"):
    if _p not in sys.path:
        sys.path.append(_p)

import ml_dtypes
import numpy as np

import concourse.bass as bass
import concourse.tile as tile
from concourse import bacc, mybir
from concourse.bass_utils import run_bass_kernel_spmd

F32 = mybir.dt.float32
BF16 = mybir.dt.bfloat16
P = 128
SCALE = 16.0
EPS = 1e-8

N_CORES = 8
N1 = 8192  # x1 rows (total)
N2 = 8192  # x2 rows
D = 512  # feature dim
KC = D // P  # K-chunks of the contraction dim
CGW = 1024  # output column-group width

_PROGRAM_CACHE = {}


def build_program(n1_local=N1 // N_CORES, n2=N2, d=D, cg_width=CGW):
    """Build the SPMD program one core runs. Returns the compiled Bacc."""
    kc = d // P
    m_tiles = n1_local // P  # 8 output row-tiles
    n_cgs = n2 // cg_width  # 8 output column groups
    nch = cg_width // 512  # 512-wide PSUM chunks per column group

    nc = bacc.Bacc("TRN2", target_bir_lowering=False, debug=False,
                   num_devices=N_CORES)
    x1n_in = nc.dram_tensor("x1", [n1_local, d], BF16, kind="ExternalInput")
    x1t = nc.dram_tensor("x1t", [P, kc, n1_local], BF16, kind="ExternalInput")
    x2t = nc.dram_tensor("x2t", [n_cgs, P, kc, cg_width], BF16,
                         kind="ExternalInput")
    out = nc.dram_tensor("out", [n1_local, n2], BF16, kind="ExternalOutput")

    with tile.TileContext(nc) as tc:
        with (
            tc.tile_pool(name="const", bufs=1) as const,
            tc.tile_pool(name="xt", bufs=1) as xt,
            tc.tile_pool(name="sq", bufs=2) as sqp,
            tc.tile_pool(name="tmp", bufs=4) as tmp,
            tc.tile_pool(name="stat", bufs=2) as stat,
            tc.tile_pool(name="nrm", bufs=2) as nrmp,
            tc.tile_pool(name="inv", bufs=3) as invp,
            tc.tile_pool(name="outs", bufs=4) as outs,
            tc.tile_pool(name="pso", bufs=3, space="PSUM") as pso,
            tc.tile_pool(name="psa", bufs=2, space="PSUM") as psa,
        ):
            ones_b = const.tile([P, P], BF16)
            nc.gpsimd.memset(ones_b[:], 1.0)

            # x1 natural rows: row = a*128 + p
            x1r = x1n_in.ap().rearrange("(a p) e -> p a e", p=P)

            x1T = xt.tile([P, kc, n1_local], BF16, tag="x1T", name="x1T")
            x2T = [xt.tile([P, kc, cg_width], BF16, tag=f"x2T_{cg}",
                           name=f"x2T_{cg}") for cg in range(n_cgs)]
            x1l = xt.tile([P, m_tiles, d], BF16, tag="x1l", name="x1l")
            invs = [None] * n_cgs
            ssums = [None] * n_cgs

            # ---- input DMAs: only 4 issued up front. The HWDGE issue
            # window is ~8 in-flight DMAs; more up-front issues head-of-line
            # block the whole scalar queue (ACT compute included) until
            # transfers retire. Later loads are issued from prep_pre, two
            # column groups ahead of use.
            nc.sync.dma_start(x1l[:], x1r)
            nc.scalar.dma_start(x2T[0][:], x2t.ap()[0])
            nc.scalar.dma_start(x1T[:], x1t.ap())
            nc.scalar.dma_start(x2T[1][:], x2t.ap()[1])

            # warm the ScalarE Sqrt table (lazy-loads 1.3us at first use)
            warm = stat.tile([P, 1], F32, tag="warm")
            nc.gpsimd.memset(warm[:], 1.0)
            nc.scalar.activation(
                warm[:], warm[:], mybir.ActivationFunctionType.Sqrt
            )

            # ---- x2 column norms: squares + K-chunk sums ------------------
            def prep_pre(cg):
                sq_t = sqp.tile([P, kc, cg_width], BF16, tag="sq2",
                                name=f"sq2_{cg}")
                nc.scalar.activation(
                    sq_t[:], x2T[cg][:],
                    mybir.ActivationFunctionType.Square,
                )
                s01 = tmp.tile([P, cg_width], BF16, tag="s01")
                s23 = tmp.tile([P, cg_width], BF16, tag="s23")
                ssum = tmp.tile([P, cg_width], BF16, tag="ssum",
                                name=f"ssum_{cg}")
                nc.vector.tensor_add(s01[:], sq_t[:, 0], sq_t[:, 1])
                nc.vector.tensor_add(s23[:], sq_t[:, 2], sq_t[:, 3])
                nc.vector.tensor_add(ssum[:], s01[:], s23[:])
                ssums[cg] = ssum

            # ---- partition reduce+broadcast, sqrt, reciprocal, and fold
            # ---- inv2 into the operand in place ---------------------------
            def prep_post(cg):
                ssum = ssums[cg]
                inv = invp.tile([P, cg_width], F32, tag="inv",
                                name=f"inv_{cg}")
                for c in range(nch):
                    cs = slice(c * 512, (c + 1) * 512)
                    ps_s = psa.tile([P, 512], F32, tag="psa",
                                    name=f"psn_{cg}_{c}")
                    nc.tensor.matmul(ps_s[:], lhsT=ones_b[:], rhs=ssum[:, cs],
                                     start=True, stop=True)
                    nrm = nrmp.tile([P, 512], F32, tag="nrm",
                                    name=f"nrm_{cg}_{c}")
                    nc.scalar.activation(
                        nrm[:], ps_s[:], mybir.ActivationFunctionType.Sqrt
                    )
                    nc.vector.reciprocal_approx_fast(inv[:, cs], nrm[:])
                invs[cg] = inv
                nc.vector.tensor_mul(
                    x2T[cg][:], x2T[cg][:],
                    inv[:, None, :].to_broadcast((P, kc, cg_width)),
                )

            def x1_stats():
                ssq1 = stat.tile([P, m_tiles], F32, tag="ssq1")
                for a in range(m_tiles):
                    sq_t = tmp.tile([P, d], BF16, tag="sq1")
                    nc.scalar.activation(
                        sq_t[:], x1l[:, a],
                        mybir.ActivationFunctionType.Square,
                        accum_out=ssq1[:, a : a + 1],
                    )
                nrm1 = stat.tile([P, m_tiles], F32, tag="nrm1")
                nc.scalar.activation(
                    nrm1[:], ssq1[:], mybir.ActivationFunctionType.Sqrt,
                    scale=1.0 / (SCALE * SCALE),
                )
                inv1 = stat.tile([P, m_tiles], F32, tag="inv1")
                nc.vector.reciprocal(inv1[:], nrm1[:])
                return inv1

            def gemm_m(cg, m):
                ps = pso.tile([P, cg_width], F32, tag="ps",
                              name=f"ps_{cg}_{m}")
                for k in range(kc):
                    for c in range(nch):
                        nc.tensor.matmul(
                            ps[:, c * 512 : (c + 1) * 512],
                            lhsT=x1T[:, k, m * P : (m + 1) * P],
                            rhs=x2T[cg][:, k, c * 512 : (c + 1) * 512],
                            start=(k == 0), stop=(k == kc - 1),
                        )
                return ps

            def drain_m(cg, m, ps):
                ot = outs.tile([P, cg_width], BF16, tag="ot",
                               name=f"ot_{cg}_{m}")
                s1 = inv1[:, m : m + 1]
                if m in (0, 1, 4):
                    nc.vector.tensor_scalar_mul(ot[:], ps[:], s1)
                else:
                    nc.scalar.activation(
                        ot[:], ps[:], mybir.ActivationFunctionType.Copy,
                        scale=s1,
                    )
                # last group's stores go on the otherwise-idle scalar ring
                # so the final drain isn't serialized behind the backlog
                eng = nc.scalar if cg == n_cgs - 1 else nc.sync
                eng.dma_start(
                    out[m * P : (m + 1) * P,
                        cg * cg_width : (cg + 1) * cg_width],
                    ot[:],
                )

            # Emission schedule. Strict per-engine FIFO => emission order is
            # engine queue order: cg0's square goes to ScalarE before the x1
            # stats burst; each prep_pre lands 2 column groups ahead and its
            # prep_post 1.5 groups ahead (m==4 hook) so the PE never waits on
            # the norm chain or the in-place operand scale.
            prep_pre(0)
            prep_post(0)
            inv1 = x1_stats()
            prep_pre(1)
            for cg in range(n_cgs):
                for m in range(m_tiles):
                    ps = gemm_m(cg, m)
                    drain_m(cg, m, ps)
                    if m == 1:
                        if cg + 2 < n_cgs:
                            nc.scalar.dma_start(x2T[cg + 2][:],
                                                x2t.ap()[cg + 2])
                        if 1 <= cg < n_cgs - 1:
                            prep_post(cg + 1)
                    if cg == 0 and m == 3:
                        prep_post(1)
                    if m == 4 and cg + 2 < n_cgs:
                        prep_pre(cg + 2)

    nc.compile()
    return nc


def _get_program():
    key = "default"
    if key not in _PROGRAM_CACHE:
        _PROGRAM_CACHE[key] = build_program()
    return _PROGRAM_CACHE[key]


def make_in_maps(x1: np.ndarray, x2: np.ndarray) -> list:
    x1 = np.asarray(x1, dtype=np.float32)
    x2 = np.asarray(x2, dtype=np.float32)
    assert x1.shape == (N1, D) and x2.shape == (N2, D), (x1.shape, x2.shape)
    x1_b = x1.astype(ml_dtypes.bfloat16)
    # K-chunk-blocked transposes: t[k*128+p, c] lands at tb[p, k, c], so
    # every load is one DMA of 128 contiguous per-partition rows.
    x2t = x2.astype(ml_dtypes.bfloat16).T
    x2tb = np.ascontiguousarray(
        x2t.reshape(KC, P, N2 // CGW, CGW).transpose(2, 1, 0, 3)
    )
    rows = N1 // N_CORES
    maps = []
    for c in range(N_CORES):
        x1c = np.ascontiguousarray(x1_b[c * rows : (c + 1) * rows])
        x1tb = np.ascontiguousarray(
            x1c.T.reshape(KC, P, rows).transpose(1, 0, 2)
        )
        maps.append({"x1": x1c, "x1t": x1tb, "x2t": x2tb})
    return maps


def kernel(x1: np.ndarray, x2: np.ndarray) -> np.ndarray:
    nc = _get_program()
    in_maps = make_in_maps(x1, x2)
    res = run_bass_kernel_spmd(nc, in_maps, core_ids=list(range(N_CORES)))
    return np.concatenate(
        [res.results[c]["out"] for c in range(N_CORES)], axis=0
    ).astype(np.float32)


if __name__ == "__main__":
    rng = np.random.default_rng(0)
    a = rng.standard_normal((N1, D), dtype=np.float32)
    b = rng.standard_normal((N2, D), dtype=np.float32)
    got = kernel(a, b)
    n1 = np.maximum(np.linalg.norm(a, axis=-1, keepdims=True), EPS)
    n2 = np.maximum(np.linalg.norm(b, axis=-1, keepdims=True), EPS)
    want = SCALE * (a / n1) @ (b / n2).T
    err = np.abs(got - want)
    rel = np.linalg.norm(got - want) / np.linalg.norm(want)
    print(f"max abs err: {err.max():.3e}  rel: {rel:.3e}")
